# revision 7
# baseline (speedup 1.0000x reference)
"""Trainium2 Bass kernel for nn_HardwareOptimizedSpikeProcessor.

Reference semantics (per timestep t):
    acc += (s_t @ (W*mask).T) * 2**scale_exp     # [B, Cout]
    spk  = acc >= 2**threshold_exp
    acc  = acc * (1 - spk)
    out[:, :, t] = spk

Strategy:
  - Data-parallel over batch: 8 cores x 8 samples.
  - The matmul contribution c[t] = s_t @ Wm.T is independent of acc, so it is
    computed as one big bf16 matmul per core ([T*B_loc, Cin] x [Cin, Cout]).
    Spikes are 0/1 and masked weights are integers in [-127, 127], both exact
    in bf16; PSUM accumulates in fp32 (sums < 2^24) => bit-exact.
  - The sequential part is a cheap elementwise scan over T=128 steps on the
    vector engine: add, compare (also the spike output), predicated reset.
  - Layouts: state acc is [cout_lo=128 partitions, (cout_hi=16, b=8)] so each
    scan step is a single full-width [128, 128] DVE op.
"""

import sys

for _p in ("/opt/trn_rl_repo",):
    if _p not in sys.path:
        sys.path.insert(0, _p)

import numpy as np
import ml_dtypes

import concourse.bass as bass
import concourse.mybir as mybir
import concourse.tile as tile
from concourse.bass_utils import run_bass_kernel_spmd

B, CIN, COUT, T = 64, 2048, 2048, 128
NCORES = 8
BLOC = B // NCORES          # samples per core
KC = CIN // 128             # 16 contraction chunks
MC = COUT // 128            # 16 output-channel chunks
TBLK = 64                   # timesteps per pipeline block
NBLK = T // TBLK
NFREE = BLOC * TBLK         # matmul free dim (b, t) = 512

_MAX_WAITS = 1


def _split_excess_waits(nc):
    """This container's walrus build accepts at most one sync-wait per
    instruction; spill extra waits onto same-engine NOPs placed before the
    offending instruction."""
    for f in nc.m.functions:
        for bb in f.blocks:
            new_list = []
            for ins in bb.instructions:
                si = ins.sync_info
                waits = list(si.on_wait) if si is not None and si.on_wait else []
                if len(waits) > _MAX_WAITS:
                    extra, keep = waits[:-_MAX_WAITS], waits[-_MAX_WAITS:]
                    for i in range(0, len(extra), _MAX_WAITS):
                        nop = mybir.InstNoOp(
                            name=f"{ins.name}-waitsplit-{i}", ins=[], outs=[]
                        )
                        nop.engine = ins.engine
                        nop.sync_info = mybir.SyncInfo(
                            on_wait=extra[i : i + _MAX_WAITS], on_update=[]
                        )
                        new_list.append(nop)
                    ins.sync_info = mybir.SyncInfo(
                        on_wait=keep,
                        on_update=list(si.on_update) if si.on_update else [],
                    )
                new_list.append(ins)
            bb.instructions[:] = new_list


BLOCKS = [(0, 32), (32, 48), (80, 32), (112, 16)]


def _build(thr: float):
    f32 = mybir.dt.float32
    bf16 = mybir.dt.bfloat16
    u8 = mybir.dt.uint8
    nc = bass.Bass()

    tmax = max(tb for _, tb in BLOCKS)

    # W^T (with 2**scale_exp folded in) [m, cin_lo, cin_hi, cout_lo]
    wt_d = nc.dram_tensor("wt", [MC, 128, KC, 128], bf16, kind="ExternalInput")
    # per-block spike tensors, each contiguous [cin_lo, cin_hi, b, tb]
    spk_ds = [
        nc.dram_tensor(f"spk{j}", [128, KC, BLOC, tb], bf16, kind="ExternalInput")
        for j, (_, tb) in enumerate(BLOCKS)
    ]
    out_d = nc.dram_tensor("out", [BLOC, COUT, T], u8, kind="ExternalOutput")
    out_v = out_d.rearrange("b (ch cl) t -> cl ch b t", cl=128)  # [128,16,8,T]

    with tile.TileContext(nc) as tc:
        with (
            tc.tile_pool(name="const", bufs=1) as const,
            tc.tile_pool(name="cpool", bufs=2) as cpool,
            tc.tile_pool(name="opool", bufs=2) as opool,
            tc.tile_pool(name="psum", bufs=2, space="PSUM") as psum,
        ):
            wt_sb = const.tile([128, MC, KC, 128], bf16)
            spk_sbs = [
                const.tile([128, KC, BLOC, tb], bf16, name=f"spk_sb{j}")
                for j, (_, tb) in enumerate(BLOCKS)
            ]
            acc = const.tile([128, 128], f32)
            zeros = const.tile([128, 128], f32)

            nc.vector.memset(acc[:], 0.0)
            nc.vector.memset(zeros[:], 0.0)
            # first block's inputs first so PE can start ASAP
            nc.sync.dma_start(spk_sbs[0][:], spk_ds[0][:])
            for m in range(MC):
                nc.sync.dma_start(wt_sb[:, m], wt_d[m])
            for j in range(1, len(BLOCKS)):
                nc.sync.dma_start(spk_sbs[j][:], spk_ds[j][:])

            deferred = None  # (ob, t0, tb) of the previous block, output late
            for j, (t0, tb) in enumerate(BLOCKS):
                nfree = BLOC * tb
                cb = cpool.tile([128, MC, BLOC * tmax], f32, tag="cblk")
                for mg in range(MC // 4):
                    ps = psum.tile([128, 4, 512], f32, tag="ps", name="ps")
                    for m4 in range(4):
                        m = mg * 4 + m4
                        for k in range(KC):
                            nc.tensor.matmul(
                                ps[:, m4, :nfree],
                                lhsT=wt_sb[:, m, k, :],
                                rhs=spk_sbs[j][:, k, :, :],
                                start=(k == 0),
                                stop=(k == KC - 1),
                            )
                    # one grouped psum->sbuf copy per 4 m-chunks
                    nc.scalar.copy(
                        cb[:, mg * 4 : (mg + 1) * 4, :nfree],
                        ps[:, :, :nfree],
                    )
                if deferred is not None:
                    _emit_output(nc, opool, out_v, *deferred)
                    deferred = None
                cb4 = cb[:, :, : BLOC * tb].rearrange(
                    "p m (b t) -> p m b t", b=BLOC
                )
                ob = opool.tile([128, tmax, 128], u8, tag="oblk")
                for t in range(tb):
                    nc.vector.tensor_tensor(
                        acc[:], acc[:], cb4[:, :, :, t], mybir.AluOpType.add
                    )
                    nc.vector.tensor_scalar(
                        ob[:, t, :], acc[:], thr, None, mybir.AluOpType.is_ge
                    )
                    nc.vector.copy_predicated(acc[:], ob[:, t, :], zeros[:])
                deferred = (ob, t0, tb)
            _emit_output(nc, opool, out_v, *deferred)

    _split_excess_waits(nc)
    return nc


def _emit_output(nc, opool, out_v, ob, t0, tb):
    """Rearrange a finished spike block to [m, b, t] and DMA it out."""
    u8 = mybir.dt.uint8
    ob2 = opool.tile([128, MC, BLOC, tb], u8, tag="oblk2", name="ob2")
    nc.scalar.copy(
        ob2[:],
        ob[:, :tb, :].rearrange("p t (m b) -> p m b t", b=BLOC),
    )
    for b in range(BLOC):
        nc.sync.dma_start(out_v[:, :, b, t0 : t0 + tb], ob2[:, :, b, :])


def _prep_inputs(spikes, weights, mask, scale_exp):
    wm = weights * mask  # integers <= 127, exact
    scale = np.exp2(scale_exp.astype(np.float64)).astype(np.float32)
    wm = (wm * scale[:, None]).astype(np.float32)  # fold power-of-2 scale in
    # [cout, cin] -> W^T [cin, cout] -> [m, cin_lo, cin_hi, cout_lo]
    wt = (
        np.ascontiguousarray(
            wm.T.reshape(KC, 128, MC, 128).transpose(2, 1, 0, 3)
        ).astype(ml_dtypes.bfloat16)
    )
    spk_cores = []
    for i in range(NCORES):
        s = spikes[i * BLOC : (i + 1) * BLOC]  # [b, cin, t]
        a = s.transpose(1, 0, 2).reshape(KC, 128, BLOC, T).transpose(1, 0, 2, 3)
        a = np.ascontiguousarray(a).astype(ml_dtypes.bfloat16)
        blks = {
            f"spk{j}": np.ascontiguousarray(a[:, :, :, t0 : t0 + tb])
            for j, (t0, tb) in enumerate(BLOCKS)
        }
        spk_cores.append(blks)
    return wt, spk_cores


_CACHE = {}


def _get_program(thr: float):
    if thr not in _CACHE:
        _CACHE[thr] = _build(thr)
    return _CACHE[thr]


def kernel(spikes, weights, mask, scale_exp, threshold_exp, **run_kwargs):
    thr = float(2.0 ** int(np.asarray(threshold_exp)))
    nc = _get_program(thr)
    wt, spk_cores = _prep_inputs(
        np.asarray(spikes, dtype=np.float32),
        np.asarray(weights, dtype=np.float32),
        np.asarray(mask, dtype=np.float32),
        np.asarray(scale_exp),
    )
    in_maps = [{"wt": wt, **spk_cores[i]} for i in range(NCORES)]
    res = run_bass_kernel_spmd(
        nc, in_maps, core_ids=list(range(NCORES)), **run_kwargs
    )
    outs = [
        np.asarray(res.results[i]["out"]).astype(np.float32)
        for i in range(NCORES)
    ]
    full = np.concatenate(outs, axis=0)  # [B, Cout, T]
    if run_kwargs:
        return full, res
    return full


# revision 8
# speedup vs baseline: 1.5387x; 1.5387x over previous
"""Trainium2 Bass kernel for nn_HardwareOptimizedSpikeProcessor.

Reference semantics (per timestep t):
    acc += (s_t @ (W*mask).T) * 2**scale_exp     # [B, Cout]
    spk  = acc >= 2**threshold_exp
    acc  = acc * (1 - spk)
    out[:, :, t] = spk

Strategy:
  - Data-parallel over batch: 8 cores x 8 samples.
  - The matmul contribution c[t] = s_t @ Wm.T is independent of acc, so it is
    computed as one big bf16 matmul per core ([T*B_loc, Cin] x [Cin, Cout]).
    Spikes are 0/1 and masked weights are integers in [-127, 127], both exact
    in bf16; PSUM accumulates in fp32 (sums < 2^24) => bit-exact.
  - The sequential part is a cheap elementwise scan over T=128 steps on the
    vector engine: add, compare (also the spike output), predicated reset.
  - Layouts: state acc is [cout_lo=128 partitions, (cout_hi=16, b=8)] so each
    scan step is a single full-width [128, 128] DVE op.
"""

import sys

for _p in ("/opt/trn_rl_repo",):
    if _p not in sys.path:
        sys.path.insert(0, _p)

import numpy as np
import ml_dtypes

import concourse.bass as bass
import concourse.mybir as mybir
import concourse.tile as tile
from concourse.bass_utils import run_bass_kernel_spmd

B, CIN, COUT, T = 64, 2048, 2048, 128
NCORES = 8
BLOC = B // NCORES          # samples per core
KC = CIN // 128             # 16 contraction chunks
MC = COUT // 128            # 16 output-channel chunks
TBLK = 64                   # timesteps per pipeline block
NBLK = T // TBLK
NFREE = BLOC * TBLK         # matmul free dim (b, t) = 512

_MAX_WAITS = 1


def _split_excess_waits(nc):
    """This container's walrus build accepts at most one sync-wait per
    instruction; spill extra waits onto same-engine NOPs placed before the
    offending instruction."""
    for f in nc.m.functions:
        for bb in f.blocks:
            new_list = []
            for ins in bb.instructions:
                si = ins.sync_info
                waits = list(si.on_wait) if si is not None and si.on_wait else []
                if len(waits) > _MAX_WAITS:
                    extra, keep = waits[:-_MAX_WAITS], waits[-_MAX_WAITS:]
                    for i in range(0, len(extra), _MAX_WAITS):
                        nop = mybir.InstNoOp(
                            name=f"{ins.name}-waitsplit-{i}", ins=[], outs=[]
                        )
                        nop.engine = ins.engine
                        nop.sync_info = mybir.SyncInfo(
                            on_wait=extra[i : i + _MAX_WAITS], on_update=[]
                        )
                        new_list.append(nop)
                    ins.sync_info = mybir.SyncInfo(
                        on_wait=keep,
                        on_update=list(si.on_update) if si.on_update else [],
                    )
                new_list.append(ins)
            bb.instructions[:] = new_list


BLOCKS = [(0, 32), (32, 48), (80, 32), (112, 16)]


def _build(thr: float):
    f32 = mybir.dt.float32
    bf16 = mybir.dt.bfloat16
    u8 = mybir.dt.uint8
    nc = bass.Bass()

    tmax = max(tb for _, tb in BLOCKS)

    # W^T (with 2**scale_exp folded in) [m, cin_lo, cin_hi, cout_lo]
    wt_d = nc.dram_tensor("wt", [MC, 128, KC, 128], bf16, kind="ExternalInput")
    # per-block spike tensors, each contiguous [cin_lo, cin_hi, b, tb]
    spk_ds = [
        nc.dram_tensor(f"spk{j}", [128, KC, BLOC, tb], bf16, kind="ExternalInput")
        for j, (_, tb) in enumerate(BLOCKS)
    ]
    out_ds = [
        nc.dram_tensor(f"out{j}", [128, tb, 128], u8, kind="ExternalOutput")
        for j, (_, tb) in enumerate(BLOCKS)
    ]

    with tile.TileContext(nc) as tc:
        with (
            tc.tile_pool(name="const", bufs=1) as const,
            tc.tile_pool(name="cpool", bufs=2) as cpool,
            tc.tile_pool(name="opool", bufs=2) as opool,
            tc.tile_pool(name="psum", bufs=2, space="PSUM") as psum,
        ):
            wt_sb = const.tile([128, MC, KC, 128], bf16)
            spk_sbs = [
                const.tile([128, KC, BLOC, tb], bf16, name=f"spk_sb{j}")
                for j, (_, tb) in enumerate(BLOCKS)
            ]
            acc = const.tile([128, 128], f32)
            zeros = const.tile([128, 128], f32)

            nc.vector.memset(acc[:], 0.0)
            nc.vector.memset(zeros[:], 0.0)
            # first block's inputs first so PE can start ASAP
            nc.sync.dma_start(spk_sbs[0][:], spk_ds[0][:])
            for m in range(MC):
                nc.sync.dma_start(wt_sb[:, m], wt_d[m])
            for j in range(1, len(BLOCKS)):
                nc.sync.dma_start(spk_sbs[j][:], spk_ds[j][:])

            for j, (t0, tb) in enumerate(BLOCKS):
                nfree = BLOC * tb
                cb = cpool.tile([128, MC, BLOC * tmax], f32, tag="cblk")
                for mg in range(MC // 4):
                    ps = psum.tile([128, 4, 512], f32, tag="ps", name="ps")
                    for m4 in range(4):
                        m = mg * 4 + m4
                        for k in range(KC):
                            nc.tensor.matmul(
                                ps[:, m4, :nfree],
                                lhsT=wt_sb[:, m, k, :],
                                rhs=spk_sbs[j][:, k, :, :],
                                start=(k == 0),
                                stop=(k == KC - 1),
                            )
                    # one grouped psum->sbuf copy per 4 m-chunks
                    nc.scalar.copy(
                        cb[:, mg * 4 : (mg + 1) * 4, :nfree],
                        ps[:, :, :nfree],
                    )
                cb4 = cb[:, :, : BLOC * tb].rearrange(
                    "p m (b t) -> p m b t", b=BLOC
                )
                ob = opool.tile([128, tmax, 128], u8, tag="oblk")
                for t in range(tb):
                    nc.vector.tensor_tensor(
                        acc[:], acc[:], cb4[:, :, :, t], mybir.AluOpType.add
                    )
                    nc.vector.tensor_scalar(
                        ob[:, t, :], acc[:], thr, None, mybir.AluOpType.is_ge
                    )
                    nc.vector.copy_predicated(acc[:], ob[:, t, :], zeros[:])
                nc.sync.dma_start(out_ds[j][:], ob[:, :tb, :])

    _split_excess_waits(nc)
    return nc


def _prep_inputs(spikes, weights, mask, scale_exp):
    wm = weights * mask  # integers <= 127, exact
    scale = np.exp2(scale_exp.astype(np.float64)).astype(np.float32)
    wm = (wm * scale[:, None]).astype(np.float32)  # fold power-of-2 scale in
    # [cout, cin] -> W^T [cin, cout] -> [m, cin_lo, cin_hi, cout_lo]
    wt = (
        np.ascontiguousarray(
            wm.T.reshape(KC, 128, MC, 128).transpose(2, 1, 0, 3)
        ).astype(ml_dtypes.bfloat16)
    )
    spk_cores = []
    for i in range(NCORES):
        s = spikes[i * BLOC : (i + 1) * BLOC]  # [b, cin, t]
        a = s.transpose(1, 0, 2).reshape(KC, 128, BLOC, T).transpose(1, 0, 2, 3)
        a = np.ascontiguousarray(a).astype(ml_dtypes.bfloat16)
        blks = {
            f"spk{j}": np.ascontiguousarray(a[:, :, :, t0 : t0 + tb])
            for j, (t0, tb) in enumerate(BLOCKS)
        }
        spk_cores.append(blks)
    return wt, spk_cores


_CACHE = {}


def _get_program(thr: float):
    if thr not in _CACHE:
        _CACHE[thr] = _build(thr)
    return _CACHE[thr]


def kernel(spikes, weights, mask, scale_exp, threshold_exp, **run_kwargs):
    thr = float(2.0 ** int(np.asarray(threshold_exp)))
    nc = _get_program(thr)
    wt, spk_cores = _prep_inputs(
        np.asarray(spikes, dtype=np.float32),
        np.asarray(weights, dtype=np.float32),
        np.asarray(mask, dtype=np.float32),
        np.asarray(scale_exp),
    )
    in_maps = [{"wt": wt, **spk_cores[i]} for i in range(NCORES)]
    res = run_bass_kernel_spmd(
        nc, in_maps, core_ids=list(range(NCORES)), **run_kwargs
    )
    outs = []
    for i in range(NCORES):
        blks = []
        for j, (t0, tb) in enumerate(BLOCKS):
            a = np.asarray(res.results[i][f"out{j}"])  # [cl, t, m*8+b]
            blks.append(a.reshape(128, tb, MC, BLOC))
        a = np.concatenate(blks, axis=1)  # [cl, T, m, b]
        # -> [b, (m cl), t]
        a = a.transpose(3, 2, 0, 1).reshape(BLOC, COUT, T)
        outs.append(a)
    full = np.concatenate(outs, axis=0).astype(np.float32)  # [B, Cout, T]
    if run_kwargs:
        return full, res
    return full


# revision 9
# speedup vs baseline: 1.5762x; 1.0244x over previous
"""Trainium2 Bass kernel for nn_HardwareOptimizedSpikeProcessor.

Reference semantics (per timestep t):
    acc += (s_t @ (W*mask).T) * 2**scale_exp     # [B, Cout]
    spk  = acc >= 2**threshold_exp
    acc  = acc * (1 - spk)
    out[:, :, t] = spk

Strategy:
  - Data-parallel over batch: 8 cores x 8 samples.
  - The matmul contribution c[t] = s_t @ Wm.T is independent of acc, so it is
    computed as one big bf16 matmul per core ([T*B_loc, Cin] x [Cin, Cout]).
    Spikes are 0/1 and masked weights are integers in [-127, 127], both exact
    in bf16; PSUM accumulates in fp32 (sums < 2^24) => bit-exact.
  - The sequential part is a cheap elementwise scan over T=128 steps on the
    vector engine: add, compare (also the spike output), predicated reset.
  - Layouts: state acc is [cout_lo=128 partitions, (cout_hi=16, b=8)] so each
    scan step is a single full-width [128, 128] DVE op.
"""

import sys

for _p in ("/opt/trn_rl_repo",):
    if _p not in sys.path:
        sys.path.insert(0, _p)

import numpy as np
import ml_dtypes

import concourse.bass as bass
import concourse.mybir as mybir
import concourse.tile as tile
from concourse.bass_utils import run_bass_kernel_spmd

B, CIN, COUT, T = 64, 2048, 2048, 128
NCORES = 8
BLOC = B // NCORES          # samples per core
KC = CIN // 128             # 16 contraction chunks
MC = COUT // 128            # 16 output-channel chunks
TBLK = 64                   # timesteps per pipeline block
NBLK = T // TBLK
NFREE = BLOC * TBLK         # matmul free dim (b, t) = 512

_MAX_WAITS = 1


def _split_excess_waits(nc):
    """This container's walrus build accepts at most one sync-wait per
    instruction; spill extra waits onto same-engine NOPs placed before the
    offending instruction."""
    for f in nc.m.functions:
        for bb in f.blocks:
            new_list = []
            for ins in bb.instructions:
                si = ins.sync_info
                waits = list(si.on_wait) if si is not None and si.on_wait else []
                if len(waits) > _MAX_WAITS:
                    extra, keep = waits[:-_MAX_WAITS], waits[-_MAX_WAITS:]
                    for i in range(0, len(extra), _MAX_WAITS):
                        nop = mybir.InstNoOp(
                            name=f"{ins.name}-waitsplit-{i}", ins=[], outs=[]
                        )
                        nop.engine = ins.engine
                        nop.sync_info = mybir.SyncInfo(
                            on_wait=extra[i : i + _MAX_WAITS], on_update=[]
                        )
                        new_list.append(nop)
                    ins.sync_info = mybir.SyncInfo(
                        on_wait=keep,
                        on_update=list(si.on_update) if si.on_update else [],
                    )
                new_list.append(ins)
            bb.instructions[:] = new_list


BLOCKS = [(0, 32), (32, 40), (72, 40), (112, 16)]


def _build(thr: float):
    f32 = mybir.dt.float32
    bf16 = mybir.dt.bfloat16
    u8 = mybir.dt.uint8
    nc = bass.Bass()

    tmax = max(tb for _, tb in BLOCKS)

    # W^T (with 2**scale_exp folded in) [m, cin_lo, cin_hi, cout_lo]
    wt_d = nc.dram_tensor("wt", [MC, 128, KC, 128], bf16, kind="ExternalInput")
    # per-block spike tensors, each contiguous [cin_lo, cin_hi, b, tb]
    spk_ds = [
        nc.dram_tensor(f"spk{j}", [128, KC, BLOC, tb], bf16, kind="ExternalInput")
        for j, (_, tb) in enumerate(BLOCKS)
    ]
    out_ds = [
        nc.dram_tensor(f"out{j}", [128, tb, 128], u8, kind="ExternalOutput")
        for j, (_, tb) in enumerate(BLOCKS)
    ]

    with tile.TileContext(nc) as tc:
        with (
            tc.tile_pool(name="const", bufs=1) as const,
            tc.tile_pool(name="cpool", bufs=3) as cpool,
            tc.tile_pool(name="opool", bufs=2) as opool,
            tc.tile_pool(name="psum", bufs=2, space="PSUM") as psum,
        ):
            wt_sb = const.tile([128, MC, KC, 128], bf16)
            spk_sbs = [
                const.tile([128, KC, BLOC, tb], bf16, name=f"spk_sb{j}")
                for j, (_, tb) in enumerate(BLOCKS)
            ]
            acc = const.tile([128, 128], f32)
            zeros = const.tile([128, 128], f32)

            nc.vector.memset(acc[:], 0.0)
            nc.vector.memset(zeros[:], 0.0)
            # first block's inputs first so PE can start ASAP
            nc.sync.dma_start(spk_sbs[0][:], spk_ds[0][:])
            for m in range(MC):
                nc.sync.dma_start(wt_sb[:, m], wt_d[m])
            for j in range(1, len(BLOCKS)):
                nc.sync.dma_start(spk_sbs[j][:], spk_ds[j][:])

            for j, (t0, tb) in enumerate(BLOCKS):
                nfree = BLOC * tb
                cb = cpool.tile([128, MC, BLOC * tmax], f32, tag="cblk")
                for mg in range(MC // 4):
                    ps = psum.tile([128, 4, 512], f32, tag="ps", name="ps")
                    for m4 in range(4):
                        m = mg * 4 + m4
                        for k in range(KC):
                            nc.tensor.matmul(
                                ps[:, m4, :nfree],
                                lhsT=wt_sb[:, m, k, :],
                                rhs=spk_sbs[j][:, k, :, :],
                                start=(k == 0),
                                stop=(k == KC - 1),
                            )
                    # one grouped psum->sbuf copy per 4 m-chunks
                    nc.scalar.copy(
                        cb[:, mg * 4 : (mg + 1) * 4, :nfree],
                        ps[:, :, :nfree],
                    )
                cb4 = cb[:, :, : BLOC * tb].rearrange(
                    "p m (b t) -> p m b t", b=BLOC
                )
                ob = opool.tile([128, tmax, 128], u8, tag="oblk")
                for t in range(tb):
                    nc.vector.tensor_tensor(
                        acc[:], acc[:], cb4[:, :, :, t], mybir.AluOpType.add
                    )
                    nc.vector.tensor_scalar(
                        ob[:, t, :], acc[:], thr, None, mybir.AluOpType.is_ge
                    )
                    nc.vector.copy_predicated(acc[:], ob[:, t, :], zeros[:])
                nc.sync.dma_start(out_ds[j][:], ob[:, :tb, :])

    _split_excess_waits(nc)
    return nc


def _prep_inputs(spikes, weights, mask, scale_exp):
    wm = weights * mask  # integers <= 127, exact
    scale = np.exp2(scale_exp.astype(np.float64)).astype(np.float32)
    wm = (wm * scale[:, None]).astype(np.float32)  # fold power-of-2 scale in
    # [cout, cin] -> W^T [cin, cout] -> [m, cin_lo, cin_hi, cout_lo]
    wt = (
        np.ascontiguousarray(
            wm.T.reshape(KC, 128, MC, 128).transpose(2, 1, 0, 3)
        ).astype(ml_dtypes.bfloat16)
    )
    spk_cores = []
    for i in range(NCORES):
        s = spikes[i * BLOC : (i + 1) * BLOC]  # [b, cin, t]
        a = s.transpose(1, 0, 2).reshape(KC, 128, BLOC, T).transpose(1, 0, 2, 3)
        a = np.ascontiguousarray(a).astype(ml_dtypes.bfloat16)
        blks = {
            f"spk{j}": np.ascontiguousarray(a[:, :, :, t0 : t0 + tb])
            for j, (t0, tb) in enumerate(BLOCKS)
        }
        spk_cores.append(blks)
    return wt, spk_cores


_CACHE = {}


def _get_program(thr: float):
    if thr not in _CACHE:
        _CACHE[thr] = _build(thr)
    return _CACHE[thr]


def kernel(spikes, weights, mask, scale_exp, threshold_exp, **run_kwargs):
    thr = float(2.0 ** int(np.asarray(threshold_exp)))
    nc = _get_program(thr)
    wt, spk_cores = _prep_inputs(
        np.asarray(spikes, dtype=np.float32),
        np.asarray(weights, dtype=np.float32),
        np.asarray(mask, dtype=np.float32),
        np.asarray(scale_exp),
    )
    in_maps = [{"wt": wt, **spk_cores[i]} for i in range(NCORES)]
    res = run_bass_kernel_spmd(
        nc, in_maps, core_ids=list(range(NCORES)), **run_kwargs
    )
    outs = []
    for i in range(NCORES):
        blks = []
        for j, (t0, tb) in enumerate(BLOCKS):
            a = np.asarray(res.results[i][f"out{j}"])  # [cl, t, m*8+b]
            blks.append(a.reshape(128, tb, MC, BLOC))
        a = np.concatenate(blks, axis=1)  # [cl, T, m, b]
        # -> [b, (m cl), t]
        a = a.transpose(3, 2, 0, 1).reshape(BLOC, COUT, T)
        outs.append(a)
    full = np.concatenate(outs, axis=0).astype(np.float32)  # [B, Cout, T]
    if run_kwargs:
        return full, res
    return full


# revision 10
# speedup vs baseline: 1.5927x; 1.0105x over previous
"""Trainium2 Bass kernel for nn_HardwareOptimizedSpikeProcessor.

Reference semantics (per timestep t):
    acc += (s_t @ (W*mask).T) * 2**scale_exp     # [B, Cout]
    spk  = acc >= 2**threshold_exp
    acc  = acc * (1 - spk)
    out[:, :, t] = spk

Strategy:
  - Data-parallel over batch: 8 cores x 8 samples.
  - The matmul contribution c[t] = s_t @ Wm.T is independent of acc, so it is
    computed as one big bf16 matmul per core ([T*B_loc, Cin] x [Cin, Cout]).
    Spikes are 0/1 and masked weights are integers in [-127, 127], both exact
    in bf16; PSUM accumulates in fp32 (sums < 2^24) => bit-exact.
  - The sequential part is a cheap elementwise scan over T=128 steps on the
    vector engine: add, compare (also the spike output), predicated reset.
  - Layouts: state acc is [cout_lo=128 partitions, (cout_hi=16, b=8)] so each
    scan step is a single full-width [128, 128] DVE op.
"""

import sys

for _p in ("/opt/trn_rl_repo",):
    if _p not in sys.path:
        sys.path.insert(0, _p)

import numpy as np
import ml_dtypes

import concourse.bass as bass
import concourse.mybir as mybir
import concourse.tile as tile
from concourse.bass_utils import run_bass_kernel_spmd

B, CIN, COUT, T = 64, 2048, 2048, 128
NCORES = 8
BLOC = B // NCORES          # samples per core
KC = CIN // 128             # 16 contraction chunks
MC = COUT // 128            # 16 output-channel chunks
TBLK = 64                   # timesteps per pipeline block
NBLK = T // TBLK
NFREE = BLOC * TBLK         # matmul free dim (b, t) = 512

_MAX_WAITS = 1


def _split_excess_waits(nc):
    """This container's walrus build accepts at most one sync-wait per
    instruction; spill extra waits onto same-engine NOPs placed before the
    offending instruction."""
    for f in nc.m.functions:
        for bb in f.blocks:
            new_list = []
            for ins in bb.instructions:
                si = ins.sync_info
                waits = list(si.on_wait) if si is not None and si.on_wait else []
                if len(waits) > _MAX_WAITS:
                    extra, keep = waits[:-_MAX_WAITS], waits[-_MAX_WAITS:]
                    for i in range(0, len(extra), _MAX_WAITS):
                        nop = mybir.InstNoOp(
                            name=f"{ins.name}-waitsplit-{i}", ins=[], outs=[]
                        )
                        nop.engine = ins.engine
                        nop.sync_info = mybir.SyncInfo(
                            on_wait=extra[i : i + _MAX_WAITS], on_update=[]
                        )
                        new_list.append(nop)
                    ins.sync_info = mybir.SyncInfo(
                        on_wait=keep,
                        on_update=list(si.on_update) if si.on_update else [],
                    )
                new_list.append(ins)
            bb.instructions[:] = new_list


BLOCKS = [(0, 32), (32, 40), (72, 40), (112, 16)]


def _build(thr: float):
    f32 = mybir.dt.float32
    bf16 = mybir.dt.bfloat16
    u8 = mybir.dt.uint8
    nc = bass.Bass()

    tmax = max(tb for _, tb in BLOCKS)

    # W^T (with 2**scale_exp folded in) [m, cin_lo, cin_hi, cout_lo]
    wt_d = nc.dram_tensor("wt", [MC, 128, KC, 128], bf16, kind="ExternalInput")
    # per-block spike tensors, each contiguous [cin_lo, cin_hi, b, tb]
    spk_ds = [
        nc.dram_tensor(f"spk{j}", [128, KC, BLOC, tb], bf16, kind="ExternalInput")
        for j, (_, tb) in enumerate(BLOCKS)
    ]
    out_ds = [
        nc.dram_tensor(f"out{j}", [128, tb, 128], u8, kind="ExternalOutput")
        for j, (_, tb) in enumerate(BLOCKS)
    ]

    with tile.TileContext(nc) as tc:
        with (
            tc.tile_pool(name="const", bufs=1) as const,
            tc.tile_pool(name="cpool", bufs=3) as cpool,
            tc.tile_pool(name="opool", bufs=2) as opool,
            tc.tile_pool(name="psum", bufs=2, space="PSUM") as psum,
        ):
            wt_sb = const.tile([128, MC, KC, 128], bf16)
            spk_sbs = [
                const.tile([128, KC, BLOC, tb], bf16, name=f"spk_sb{j}")
                for j, (_, tb) in enumerate(BLOCKS)
            ]
            acc = const.tile([128, 128], f32)
            zeros = const.tile([128, 128], f32)

            nc.vector.memset(acc[:], 0.0)
            nc.vector.memset(zeros[:], 0.0)
            # first block's inputs first so PE can start ASAP
            nc.sync.dma_start(spk_sbs[0][:], spk_ds[0][:])
            for m in range(MC):
                nc.sync.dma_start(wt_sb[:, m], wt_d[m])
            for j in range(1, len(BLOCKS)):
                nc.sync.dma_start(spk_sbs[j][:], spk_ds[j][:])

            for j, (t0, tb) in enumerate(BLOCKS):
                nfree = BLOC * tb
                cb = cpool.tile([128, BLOC, MC, tmax], f32, tag="cblk")
                for mg in range(MC // 4):
                    ps = psum.tile([128, 4, 512], f32, tag="ps", name="ps")
                    for m4 in range(4):
                        m = mg * 4 + m4
                        for k in range(KC):
                            nc.tensor.matmul(
                                ps[:, m4, :nfree],
                                lhsT=wt_sb[:, m, k, :],
                                rhs=spk_sbs[j][:, k, :, :],
                                start=(k == 0),
                                stop=(k == KC - 1),
                            )
                    # one grouped psum->sbuf copy per 4 m-chunks,
                    # iterated (m4, b, t) to land b-outer in cb
                    nc.scalar.copy(
                        cb[:, :, mg * 4 : (mg + 1) * 4, :tb].rearrange(
                            "p b m t -> p m b t"
                        ),
                        ps[:, :, :nfree].rearrange(
                            "p m (b t) -> p m b t", b=BLOC
                        ),
                    )
                cb4 = cb[:, :, :, :tb]  # [128, b, m, t]
                ob = opool.tile([128, tmax, 128], u8, tag="oblk")
                for t in range(tb):
                    nc.vector.tensor_tensor(
                        acc[:], acc[:], cb4[:, :, :, t], mybir.AluOpType.add
                    )
                    # keep = (acc < thr); spikes recovered on host as 1-keep
                    nc.vector.tensor_scalar(
                        ob[:, t, :], acc[:], thr, None, mybir.AluOpType.is_lt
                    )
                    nc.vector.tensor_tensor(
                        acc[:], acc[:], ob[:, t, :], mybir.AluOpType.mult
                    )
                nc.sync.dma_start(out_ds[j][:], ob[:, :tb, :])

    _split_excess_waits(nc)
    return nc


def _prep_inputs(spikes, weights, mask, scale_exp):
    wm = weights * mask  # integers <= 127, exact
    scale = np.exp2(scale_exp.astype(np.float64)).astype(np.float32)
    wm = (wm * scale[:, None]).astype(np.float32)  # fold power-of-2 scale in
    # [cout, cin] -> W^T [cin, cout] -> [m, cin_lo, cin_hi, cout_lo]
    wt = (
        np.ascontiguousarray(
            wm.T.reshape(KC, 128, MC, 128).transpose(2, 1, 0, 3)
        ).astype(ml_dtypes.bfloat16)
    )
    spk_cores = []
    for i in range(NCORES):
        s = spikes[i * BLOC : (i + 1) * BLOC]  # [b, cin, t]
        a = s.transpose(1, 0, 2).reshape(KC, 128, BLOC, T).transpose(1, 0, 2, 3)
        a = np.ascontiguousarray(a).astype(ml_dtypes.bfloat16)
        blks = {
            f"spk{j}": np.ascontiguousarray(a[:, :, :, t0 : t0 + tb])
            for j, (t0, tb) in enumerate(BLOCKS)
        }
        spk_cores.append(blks)
    return wt, spk_cores


_CACHE = {}


def _get_program(thr: float):
    if thr not in _CACHE:
        _CACHE[thr] = _build(thr)
    return _CACHE[thr]


def kernel(spikes, weights, mask, scale_exp, threshold_exp, **run_kwargs):
    thr = float(2.0 ** int(np.asarray(threshold_exp)))
    nc = _get_program(thr)
    wt, spk_cores = _prep_inputs(
        np.asarray(spikes, dtype=np.float32),
        np.asarray(weights, dtype=np.float32),
        np.asarray(mask, dtype=np.float32),
        np.asarray(scale_exp),
    )
    in_maps = [{"wt": wt, **spk_cores[i]} for i in range(NCORES)]
    res = run_bass_kernel_spmd(
        nc, in_maps, core_ids=list(range(NCORES)), **run_kwargs
    )
    outs = []
    for i in range(NCORES):
        blks = []
        for j, (t0, tb) in enumerate(BLOCKS):
            a = np.asarray(res.results[i][f"out{j}"])  # [cl, t, b*16+m]
            blks.append(a.reshape(128, tb, BLOC, MC))
        a = np.concatenate(blks, axis=1)  # [cl, T, b, m]
        # keep -> spike, and -> [b, (m cl), t]
        a = (1 - a).transpose(2, 3, 0, 1).reshape(BLOC, COUT, T)
        outs.append(a)
    full = np.concatenate(outs, axis=0).astype(np.float32)  # [B, Cout, T]
    if run_kwargs:
        return full, res
    return full
